# revision 14
# baseline (speedup 1.0000x reference)
"""Trainium2 Bass kernel for nn_Contextualizer (sparse_attention).

Per-core computation (data-parallel over batch B=8 across 8 NeuronCores):
    x0, x1 = split(x, 2, axis=-1)            # [N, D] each, N=2048, D=1024
    xn = x0 / sqrt(sum(x0^2, -1) + eps)      # row-normalize
    cosim = xn @ xn.T                        # [N, N], symmetric
    attn = tril(spatial_proj) * cosim
    out = (attn @ x0) * x1

Only lower-triangle tiles are computed (~half the matmul work).  Gram
formulation: G = x0 @ x0.T, with the two 1/norm factors folded into the
existing elementwise stages:
    attnT[m, n] = maskT[m, n] * G[m, n] * s[m]      (mask stage)
    out[n, d]   = (ctx'[n, d] * s[n]) * x1[n, d]    (gating stage)
where s = 1/sqrt(sum x0^2 + eps), maskT = tril(spatial_proj).T (host),
ctx'[n, d] = sum_m attnT[m, n] * x0[m, d].

All device traffic is bf16 (error budget 2e-2; measured ~4e-3): the host
pre-splits x into x0/x1, pre-transposes x0 into x0T [D, N], and converts
the masked tril(spatial_proj).T -- so the kernel does no on-device
transposes, casts, or PSUM copies.  The output is returned bf16 and
upcast on the host.  Within each strip the edge tiles above the diagonal
are skipped column-wise.  matmul2 runs one F=1024 accumulation chain per
output tile (halves instruction + weight-load count vs 2x512).
"""

import numpy as np

B = 8
N = 2048
D = 1024
P = 128
NT = N // P      # 16 row tiles
DK = D // P      # 8 contraction tiles for matmul1
NJ_W = 512       # matmul1 free-dim chunk (n)
NJS = N // NJ_W  # 4
TPS = NJ_W // P  # 4 n-tiles per strip
EPS = 1e-8

_NC_CACHE = {}


def _build():
    from concourse import bacc, mybir
    from concourse.tile import TileContext

    f32 = mybir.dt.float32
    bf16 = mybir.dt.bfloat16
    AF = mybir.ActivationFunctionType
    OP = mybir.AluOpType

    nc = bacc.Bacc("TRN2", debug=False, num_devices=B)
    x0_ext = nc.declare_dram_parameter("x0", [N, D], bf16, isOutput=False)
    x1_ext = nc.declare_dram_parameter("x1", [N, D], bf16, isOutput=False)
    x0t_ext = nc.declare_dram_parameter("x0t", [D, N], bf16, isOutput=False)
    m_ext = nc.declare_dram_parameter("maskT", [N, N], bf16, isOutput=False)
    out_ext = nc.declare_dram_parameter("out", [N, D], bf16, isOutput=True)

    with TileContext(nc) as tc:
        with (
            tc.tile_pool(name="big", bufs=1) as big,
            tc.tile_pool(name="astrip", bufs=2) as astrip,
            tc.tile_pool(name="maskp", bufs=14) as maskp,
            tc.tile_pool(name="gio", bufs=2) as gio,
            tc.tile_pool(name="outp", bufs=2) as outp,
            tc.tile_pool(name="pa", bufs=5, space="PSUM") as pa,
            tc.tile_pool(name="pb", bufs=3, space="PSUM") as pb,
        ):
            xb = big.tile([P, NT, D], bf16)        # x0, natural [m, d]
            x0T = big.tile([P, DK, N], bf16)       # x0 transposed: [d, n]
            nrm2 = big.tile([P, NT], f32)
            scal = big.tile([P, NT], f32)
            tmpa = big.tile([P, NT], f32)
            tmpb = big.tile([P, NT], f32)
            # Shared Square-output scratch: only accum_out is consumed, and
            # Squares are serialized on the Scalar queue anyway.
            sqd = big.tile([P, D], f32)

            def load_x0t_chunk(nj, ways=1, q=None):
                """DMA x0T columns [nj*512, (nj+1)*512) from the host-side
                transposed copy; rows land as [p, dk, n].  `ways` splits
                the transfer into alternating sync/scalar pieces so the
                first dk blocks land early (startup critical path)."""
                n0 = nj * NJ_W
                dkw = DK // ways
                for w in range(ways):
                    qw = q or (nc.sync if w % 2 == 0 else nc.scalar)
                    qw.dma_start(
                        x0T[:, w * dkw : (w + 1) * dkw, n0 : n0 + NJ_W],
                        x0t_ext.ap()[
                            w * dkw * P : (w + 1) * dkw * P, n0 : n0 + NJ_W
                        ].rearrange("(c p) n -> p c n", p=P),
                    )

            def load_xb_pair(t0, q):
                q.dma_start(
                    xb[:, t0 : t0 + 2, :],
                    x0_ext.ap()[t0 * P : (t0 + 2) * P, :].rearrange(
                        "(c p) d -> p c d", p=P
                    ),
                )

            def square_pair(t0):
                for i in (t0, t0 + 1):
                    nc.scalar.activation(
                        sqd[:], xb[:, i, :], AF.Square,
                        accum_out=nrm2[:, i : i + 1],
                    )
                # scal[:, t0:t0+2] = rsqrt(nrm2 + EPS), one Newton step
                sl = slice(t0, t0 + 2)
                nc.gpsimd.tensor_scalar_add(tmpa[:, sl], nrm2[:, sl], EPS)
                nc.scalar.activation(tmpb[:, sl], tmpa[:, sl], AF.Sqrt)
                nc.vector.reciprocal(scal[:, sl], tmpb[:, sl])
                nc.gpsimd.tensor_mul(tmpb[:, sl], scal[:, sl], scal[:, sl])
                nc.gpsimd.tensor_mul(tmpb[:, sl], tmpb[:, sl], tmpa[:, sl])
                nc.gpsimd.tensor_scalar(
                    tmpb[:, sl], tmpb[:, sl], -0.5, 1.5, op0=OP.mult, op1=OP.add
                )
                nc.gpsimd.tensor_mul(scal[:, sl], scal[:, sl], tmpb[:, sl])

            def load_mask(nj, mi2):
                n0 = nj * NJ_W
                mt = maskp.tile([P, 2, NJ_W], bf16, tag="mt")
                dma_q = nc.sync if (mi2 // 2) % 2 == 0 else nc.scalar
                dma_q.dma_start(
                    mt[:],
                    m_ext.ap()[
                        mi2 * P : (mi2 + 2) * P, n0 : n0 + NJ_W
                    ].rearrange("(c p) n -> p c n", p=P),
                )
                return mt

            def phase_a(nj, pre):
                """attnT strip for n-chunk nj: tiles mi = 0..4nj+3.

                Edge tiles with mi > 4*nj start at column (mi-4*nj)*128 of
                the strip; everything left of that is above the diagonal
                and masked to zero, so it is skipped.  Mask tiles were
                DMA-issued one strip ahead (the `pre` list).
                """
                n0 = nj * NJ_W
                n_mtiles = 4 * nj + 4
                A = astrip.tile([P, NT, NJ_W], bf16, tag="A")
                for mi2 in range(0, n_mtiles, 2):
                    mt = pre.pop(0)
                    for c in range(2):
                        mi = mi2 + c
                        lo = max(0, (mi - 4 * nj) * P)
                        pcs = pa.tile([P, NJ_W], f32)
                        for dk in range(DK):
                            nc.tensor.matmul(
                                pcs[:, lo:NJ_W],
                                x0T[:, dk, mi * P : (mi + 1) * P],
                                x0T[:, dk, n0 + lo : n0 + NJ_W],
                                start=(dk == 0),
                                stop=(dk == DK - 1),
                            )
                        nc.vector.scalar_tensor_tensor(
                            out=A[:, mi, lo:NJ_W],
                            in0=pcs[:, lo:NJ_W],
                            scalar=scal[:, mi : mi + 1],
                            in1=mt[:, c, lo:NJ_W],
                            op0=OP.mult,
                            op1=OP.mult,
                        )
                return A

            def phase_b(nj, A, last=False):
                """ctx rows for n-tiles 4nj..4nj+3; scale+gate with x1; DMA."""
                for sub in range(TPS):
                    ni = TPS * nj + sub
                    off = sub * P
                    x1t = gio.tile([P, D], bf16, tag="x1t")
                    # gpsimd SWDGE: keeps the gating loads off the HWDGE
                    # queues, whose rings are FIFO'd behind mask prefetch.
                    nc.gpsimd.dma_start(
                        x1t[:], x1_ext.ap()[ni * P : (ni + 1) * P, :]
                    )
                    ot = outp.tile([P, D], bf16, tag="ot")
                    for dc in range(2):
                        pob = pb.tile([P, 512], f32, tag="pob")
                        for mi in range(ni + 1):
                            nc.tensor.matmul(
                                pob[:],
                                A[:, mi, off : off + P],
                                xb[:, mi, dc * 512 : (dc + 1) * 512],
                                start=(mi == 0),
                                stop=(mi == ni),
                            )
                        nc.vector.scalar_tensor_tensor(
                            out=ot[:, dc * 512 : (dc + 1) * 512],
                            in0=pob[:],
                            scalar=scal[:, ni : ni + 1],
                            in1=x1t[:, dc * 512 : (dc + 1) * 512],
                            op0=OP.mult,
                            op1=OP.mult,
                        )
                    if last:
                        # End-game: split stores across three queues so the
                        # final transfers don't serialize behind one ring.
                        nc.sync.dma_start(
                            out_ext.ap()[ni * P : (ni + 1) * P, 0:384],
                            ot[:, 0:384],
                        )
                        nc.scalar.dma_start(
                            out_ext.ap()[ni * P : (ni + 1) * P, 384:768],
                            ot[:, 384:768],
                        )
                        nc.gpsimd.dma_start(
                            out_ext.ap()[ni * P : (ni + 1) * P, 768:D],
                            ot[:, 768:D],
                        )
                    else:
                        qo = nc.sync if sub % 2 == 0 else nc.scalar
                        qo.dma_start(
                            out_ext.ap()[ni * P : (ni + 1) * P, :], ot[:]
                        )

            # Startup: x0T chunk 0 lands in fine-grained dk pieces so the
            # first matmul chain starts as soon as its first blocks are
            # resident; chunks 1-2 follow immediately (HWDGE rings drain
            # FIFO, so issue order == priority order).  xb tiles and
            # strip-0 masks ride behind; Squares queue last on Scalar.
            load_x0t_chunk(0, ways=4)
            load_xb_pair(0, nc.sync)
            load_xb_pair(2, nc.scalar)
            masks_cur = [load_mask(0, 0), load_mask(0, 2)]
            load_x0t_chunk(1, ways=2)
            square_pair(0)
            square_pair(2)

            prev_A = None
            for nj in range(NJS):
                masks_next = None
                if nj + 1 < NJS:
                    load_xb_pair(4 * nj + 4, nc.sync)
                    load_xb_pair(4 * nj + 6, nc.scalar)
                    masks_next = [
                        load_mask(nj + 1, mi2)
                        for mi2 in range(0, 4 * (nj + 1) + 4, 2)
                    ]
                if nj + 2 < NJS:
                    load_x0t_chunk(nj + 2, ways=2)
                A = phase_a(nj, masks_cur)
                if prev_A is not None:
                    phase_b(nj - 1, prev_A)
                if nj + 1 < NJS:
                    square_pair(4 * nj + 4)
                    square_pair(4 * nj + 6)
                masks_cur = masks_next
                prev_A = A
            phase_b(NJS - 1, prev_A, last=True)

    nc.compile()
    return nc


def _get_nc():
    if "nc" not in _NC_CACHE:
        _NC_CACHE["nc"] = _build()
    return _NC_CACHE["nc"]


def _run(x, spatial_proj, trace=False):
    import ml_dtypes
    from concourse.bass_utils import run_bass_kernel_spmd

    nc = _get_nc()
    bf = ml_dtypes.bfloat16
    x = np.asarray(x, dtype=np.float32)
    sp = np.asarray(spatial_proj, dtype=np.float32)
    x0 = np.ascontiguousarray(x[:, :, :D]).astype(bf)
    x1 = np.ascontiguousarray(x[:, :, D:]).astype(bf)
    x0t = np.ascontiguousarray(np.swapaxes(x[:, :, :D], 1, 2)).astype(bf)
    maskT = np.ascontiguousarray(np.tril(sp).T).astype(bf)
    in_maps = [
        {"x0": x0[b], "x1": x1[b], "x0t": x0t[b], "maskT": maskT}
        for b in range(B)
    ]
    res = run_bass_kernel_spmd(
        nc, in_maps, core_ids=list(range(B)), trace=trace
    )
    out = np.stack([res.results[b]["out"] for b in range(B)], axis=0)
    return out.astype(np.float32), res


def kernel(x, spatial_proj):
    out, _ = _run(x, spatial_proj, trace=False)
    return out


if __name__ == "__main__":
    rng = np.random.default_rng(0)
    x = rng.standard_normal((B, N, 2 * D), dtype=np.float32)
    sp = (rng.standard_normal((N, N), dtype=np.float32) * np.sqrt(1.0 / N)).astype(
        np.float32
    )
    out = kernel(x, sp)
    print("out shape", out.shape, out.dtype)
